# revision 3
# baseline (speedup 1.0000x reference)
"""Dense soft-MoE layer for Trainium2, expert-parallel across 8 NeuronCores.

V5 = V4 + chunk-paired first matmul: token chunks are processed in
pairs, and each w1 stationary block is loaded once and streamed against
BOTH chunks' x (halves L1 LDWEIGHTS stationary switches, ~100 ns each
on this toolchain). Holding two hT buffers forces w1 out of SBUF, so
w1 f-blocks are streamed from HBM inside the loop (ring of 4, ~32 MB
per iteration, overlapped on otherwise idle DMA). L2 stays V4-flipped
(stationary = hT token-block reused across both d-halves). The gate
softmax chain is pipelined one chunk-pair ahead.
"""
import sys

sys.path.insert(0, "/opt/trn_rl_repo")

import numpy as np
import ml_dtypes

D = 1024
F = 4096
E = 8
T = 4096
P = 128
TC = 512            # token chunk
NCH = T // TC       # 8 chunks
NSP = NCH // 2      # 4 chunk pairs
KD = D // P         # 8 k-tiles (contraction of first matmul)
KF = F // P         # 32 f-tiles (contraction of second matmul)
ND = D // P         # 8 output d-tiles
NTB = TC // P       # 4 token blocks per chunk
NDH = D // 512      # 2 d-halves
QF = 4              # w1 f-blocks per streamed quad (1 KB DMA segments)
NQ = KF // QF       # 8 quads

_cache = {}


def _build(reps: int = 1, loop_n: int = 0):
    import contextlib
    import concourse.mybir as mybir
    import concourse.tile as tile
    from concourse import bacc

    dt = mybir.dt
    AF = mybir.ActivationFunctionType
    ALU = mybir.AluOpType

    nc = bacc.Bacc(None, target_bir_lowering=False, debug=False)

    xT = nc.dram_tensor("xT", [D, T], dt.bfloat16, kind="ExternalInput")
    w1e = nc.dram_tensor("w1e", [D, F], dt.bfloat16, kind="ExternalInput")
    w2e = nc.dram_tensor("w2e", [F, D], dt.bfloat16, kind="ExternalInput")
    b1e = nc.dram_tensor("b1e", [F], dt.float32, kind="ExternalInput")
    b2r = nc.dram_tensor("b2r", [1, D], dt.bfloat16, kind="ExternalInput")
    gw = nc.dram_tensor("gw", [D, E], dt.bfloat16, kind="ExternalInput")
    gbh = nc.dram_tensor("gbh", [E, 1], dt.float32, kind="ExternalInput")
    sele = nc.dram_tensor("sele", [E, 1], dt.bfloat16, kind="ExternalInput")
    out = nc.dram_tensor("out", [T, D], dt.float16, kind="ExternalOutput")

    with tile.TileContext(nc) as tc:
        with tc.tile_pool(name="weights", bufs=1) as wpool, \
             tc.tile_pool(name="w1s", bufs=2) as w1pool, \
             tc.tile_pool(name="consts", bufs=1) as cpool, \
             tc.tile_pool(name="xin", bufs=4) as xpool, \
             tc.tile_pool(name="hbuf", bufs=2) as hpool, \
             tc.tile_pool(name="psum", bufs=4, space="PSUM") as ppool, \
             tc.tile_pool(name="py", bufs=2, space="PSUM") as pypool, \
             tc.tile_pool(name="lgt", bufs=1, space="PSUM") as lgpool, \
             tc.tile_pool(name="gdn", bufs=1, space="PSUM") as dnpool, \
             tc.tile_pool(name="small", bufs=4) as spool, \
             tc.tile_pool(name="small2", bufs=4) as s2pool, \
             tc.tile_pool(name="gate", bufs=2) as gatepool, \
             tc.tile_pool(name="outb", bufs=3) as opool:

            w1_re = w1e.rearrange("(k p) f -> p k f", p=P)
            w2_re = w2e.rearrange("(k p) d -> p k d", p=P)
            w2_sb = wpool.tile([P, KF, D], dt.bfloat16)
            for k8 in range(4):
                ks = slice(k8 * (KF // 4), (k8 + 1) * (KF // 4))
                nc.sync.dma_start(w2_sb[:, ks, :], w2_re[:, ks, :])

            b1_sb = cpool.tile([P, KF], dt.float32)
            nc.sync.dma_start(b1_sb[:], b1e.rearrange("(f p) -> p f", p=P))
            b2_sb = cpool.tile([1, D], dt.bfloat16)
            nc.sync.dma_start(b2_sb[:], b2r[:])
            gw_sb = cpool.tile([P, KD, E], dt.bfloat16)
            nc.sync.dma_start(gw_sb[:], gw.rearrange("(k p) e -> p k e", p=P))
            gbh_sb = cpool.tile([E, 1], dt.float32)
            nc.sync.dma_start(gbh_sb[:], gbh[:])
            sele_sb = cpool.tile([E, 1], dt.bfloat16)
            nc.sync.dma_start(sele_sb[:], sele[:])
            ones8 = cpool.tile([E, 1], dt.bfloat16)
            nc.any.memset(ones8[:], 1.0)
            onesr = cpool.tile([1, P], dt.bfloat16)
            nc.any.memset(onesr[:], 1.0)
            ones11 = cpool.tile([1, 1], dt.bfloat16)
            nc.any.memset(ones11[:], 1.0)

            xT_re = xT.rearrange("(k p) t -> p k t", p=P)

            def dma_x(c):
                xn = xpool.tile([P, KD, TC], dt.bfloat16, tag="x")
                nc.sync.dma_start(xn[:], xT_re[:, :, c * TC:(c + 1) * TC])
                return xn

            def emit_lg(x_sb):
                lg = lgpool.tile([E, TC], dt.float32, tag="lgt")
                for k in range(KD):
                    nc.tensor.matmul(lg[:], gw_sb[:, k, :], x_sb[:, k, :],
                                     start=(k == 0), stop=(k == KD - 1))
                return lg

            def emit_exp(lg):
                expT = s2pool.tile([E, TC], dt.bfloat16, tag="expT")
                tt = spool.tile([E, TC], dt.float32, tag="gs")
                nc.scalar.activation(tt[:], lg[:], AF.Tanh,
                                     bias=gbh_sb[:], scale=0.5)
                bm = spool.tile([E, TC], dt.float32, tag="gs")
                nc.vector.tensor_scalar(bm[:], tt[:], -1.0, 1.0,
                                        op0=ALU.mult, op1=ALU.add)
                rb = spool.tile([E, TC], dt.float32, tag="gs")
                nc.vector.reciprocal(rb[:], bm[:])
                ap1 = spool.tile([E, TC], dt.float32, tag="gs")
                nc.vector.tensor_scalar_add(ap1[:], tt[:], 1.0)
                nc.vector.tensor_mul(expT[:], ap1[:], rb[:])
                return expT

            def emit_den(expT):
                den = dnpool.tile([1, TC], dt.float32, tag="dn")
                nc.tensor.matmul(den[:], ones8[:], expT[:], start=True, stop=True)
                rec = s2pool.tile([1, TC], dt.float32, tag="rec")
                nc.vector.reciprocal(rec[:], den[:])
                return rec

            def emit_num(expT, rec):
                num = dnpool.tile([1, TC], dt.float32, tag="dn")
                nc.tensor.matmul(num[:], sele_sb[:], expT[:], start=True, stop=True)
                gcol = s2pool.tile([1, TC], dt.bfloat16, tag="gcol")
                nc.vector.tensor_mul(gcol[:], num[:], rec[:])
                return gcol

            def emit_gT(gcol):
                gT_sb = gatepool.tile([P, NTB], dt.float32, tag="gT")
                for tb in range(NTB):
                    gt = lgpool.tile([P, 1], dt.float32, tag="lgt")
                    nc.tensor.matmul(gt[:], gcol[0:1, tb * P:(tb + 1) * P],
                                     ones11[:], start=True, stop=True)
                    nc.vector.tensor_copy(gT_sb[:, tb:tb + 1], gt[:])
                return gT_sb

            def dma_w1q(q):
                # quad of 4 f-blocks: per partition each k-row is 4*128
                # contiguous bf16 = 1 KB — the efficient DMA segment size
                wt = w1pool.tile([P, KD, QF * P], dt.bfloat16, tag="w1q")
                nc.sync.dma_start(wt[:], w1_re[:, :, q * QF * P:(q + 1) * QF * P])
                return wt

            def emit_l2(c, hT, gT_sb):
                for tb in range(NTB):
                    pys = []
                    for _dh in range(NDH):
                        pyt = pypool.tile([P, 512], dt.float32, tag="py")
                        pys.append(pyt)
                    for f in range(KF):
                        hsl = hT[:, f, tb * P:(tb + 1) * P]
                        for dh in range(NDH):
                            nc.tensor.matmul(
                                pys[dh][:], hsl,
                                w2_sb[:, f, dh * 512:(dh + 1) * 512],
                                start=(f == 0), stop=False)
                    for dh in range(NDH):
                        nc.tensor.matmul(
                            pys[dh][:], onesr[:],
                            b2_sb[0:1, dh * 512:(dh + 1) * 512],
                            start=False, stop=True)
                    for dh in range(NDH):
                        ob = opool.tile([P, 512], dt.float16, tag="ob")
                        nc.vector.tensor_scalar_mul(
                            ob[:], pys[dh][:], gT_sb[:, tb:tb + 1])
                        nc.scalar.dma_start(
                            out[c * TC + tb * P:c * TC + (tb + 1) * P,
                                dh * 512:(dh + 1) * 512], ob[:])

            loop_cm = tc.For_i(0, loop_n, 1) if loop_n else contextlib.nullcontext()
            with loop_cm:
              for _rep in range(reps):
                st = {}
                for s in range(NSP):
                    c0, c1 = 2 * s, 2 * s + 1
                    if s == 0:
                        st["x0"] = dma_x(0)
                        st["x1"] = dma_x(1)
                    x0, x1 = st[f"x{c0}"], st[f"x{c1}"]
                    hT0 = hpool.tile([P, KF, TC], dt.bfloat16, tag="hT")
                    hT1 = hpool.tile([P, KF, TC], dt.bfloat16, tag="hT")

                    # gate / prefetch events keyed on f within the L1 pair
                    # (each runs between f-groups so the PE never waits on
                    # the ACT/DVE softmax chain)
                    ev = {}
                    if s == 0:
                        # boot: whole gate chain for chunks 0,1 in-body
                        lg0 = emit_lg(x0)
                        e0 = emit_exp(lg0)
                        st["e0"] = e0
                        ev[3] = lambda: st.__setitem__("r0", emit_den(st["e0"]))
                        ev[6] = lambda: st.__setitem__("g0", emit_num(st["e0"], st["r0"]))
                        ev[9] = lambda: st.__setitem__("e1", emit_exp(emit_lg(x1)))
                        ev[13] = lambda: st.__setitem__("r1", emit_den(st["e1"]))
                        ev[16] = lambda: st.__setitem__("g1", emit_num(st["e1"], st["r1"]))
                    else:
                        ev[2] = lambda: st.__setitem__(f"r{c0}", emit_den(st[f"e{c0}"]))
                        ev[5] = lambda: st.__setitem__(f"g{c0}", emit_num(st[f"e{c0}"], st[f"r{c0}"]))
                        ev[8] = lambda: st.__setitem__(f"r{c1}", emit_den(st[f"e{c1}"]))
                        ev[11] = lambda: st.__setitem__(f"g{c1}", emit_num(st[f"e{c1}"], st[f"r{c1}"]))
                    # w1 quad boot + next-pair x prefetches queue first on
                    # the sync queue; the mid-loop events then only run
                    # gate compute against long-since-landed x
                    w1t = {0: dma_w1q(0), 1: dma_w1q(1)}
                    if c1 + 1 < NCH:
                        st[f"x{c1 + 1}"] = dma_x(c1 + 1)
                    if c1 + 2 < NCH:
                        st[f"x{c1 + 2}"] = dma_x(c1 + 2)
                    if c1 + 1 < NCH:
                        def pf2(c=c1 + 1):
                            st[f"e{c}"] = emit_exp(emit_lg(st[f"x{c}"]))
                        ev[20] = pf2
                    if c1 + 2 < NCH:
                        def pf3(c=c1 + 2):
                            st[f"e{c}"] = emit_exp(emit_lg(st[f"x{c}"]))
                        ev[26] = pf3
                    for f in range(KF):
                        if f in ev:
                            ev[f]()
                        q, fo = divmod(f, QF)
                        if fo == 0 and q + 2 < NQ:
                            w1t[q + 2] = dma_w1q(q + 2)
                        wsl = w1t[q]
                        ph0 = ppool.tile([P, TC], dt.float32, tag="mm")
                        ph1 = ppool.tile([P, TC], dt.float32, tag="mm")
                        for k in range(KD):
                            nc.tensor.matmul(ph0[:],
                                             wsl[:, k, fo * P:(fo + 1) * P],
                                             x0[:, k, :],
                                             start=(k == 0), stop=(k == KD - 1))
                            nc.tensor.matmul(ph1[:],
                                             wsl[:, k, fo * P:(fo + 1) * P],
                                             x1[:, k, :],
                                             start=(k == 0), stop=(k == KD - 1))
                        nc.scalar.activation(hT0[:, f, :], ph0[:], AF.Gelu,
                                             bias=b1_sb[:, f:f + 1])
                        nc.scalar.activation(hT1[:, f, :], ph1[:], AF.Gelu,
                                             bias=b1_sb[:, f:f + 1])

                    # --- L2 per chunk (V4 flipped form) ---
                    emit_l2(c0, hT0, emit_gT(st[f"g{c0}"]))
                    emit_l2(c1, hT1, emit_gT(st[f"g{c1}"]))

    nc.compile()
    return nc


def make_in_maps(inputs, gate_w, gate_b, w1, b1, w2, b2):
    x = np.ascontiguousarray(np.asarray(inputs).reshape(-1, D))       # [T, D]
    xT16 = np.ascontiguousarray(x.T).astype(ml_dtypes.bfloat16)       # [D, T]
    gw16 = np.asarray(gate_w, dtype=ml_dtypes.bfloat16)
    gbh32 = np.asarray(gate_b, dtype=np.float32).reshape(E, 1) * 0.5

    in_maps = []
    for e in range(E):
        sele = np.zeros((E, 1), dtype=ml_dtypes.bfloat16)
        sele[e, 0] = 1.0
        in_maps.append({
            "xT": xT16,
            "w1e": np.ascontiguousarray(w1[e]).astype(ml_dtypes.bfloat16),
            "w2e": np.ascontiguousarray(w2[e]).astype(ml_dtypes.bfloat16),
            "b1e": np.asarray(b1[e], dtype=np.float32),
            "b2r": np.asarray(b2[e], dtype=ml_dtypes.bfloat16).reshape(1, D),
            "gw": gw16,
            "gbh": gbh32,
            "sele": sele,
        })
    return in_maps


def make_in_map(inputs_dict, e):
    return make_in_maps(inputs_dict["inputs"], inputs_dict["gate_w"],
                        inputs_dict["gate_b"], inputs_dict["w1"],
                        inputs_dict["b1"], inputs_dict["w2"],
                        inputs_dict["b2"])[e]


def kernel(inputs, gate_w, gate_b, w1, b1, w2, b2):
    from concourse.bass_utils import run_bass_kernel_spmd

    if "nc" not in _cache:
        _cache["nc"] = _build()
    nc = _cache["nc"]

    B, S, Dm = inputs.shape
    in_maps = make_in_maps(inputs, gate_w, gate_b, w1, b1, w2, b2)

    res = run_bass_kernel_spmd(nc, in_maps, core_ids=list(range(E)))
    _cache["last_results"] = res

    acc = res.results[0]["out"].astype(np.float64)
    for e in range(1, E):
        acc += res.results[e]["out"]
    return acc.astype(np.float32).reshape(B, S, Dm)
